# revision 24
# baseline (speedup 1.0000x reference)
"""Normalized GCN conv on 8 Trainium2 NeuronCores.

Reference computation:
    h = x @ W.T                                  # [n, d]
    (src, dst) += self loops
    deg = bincount(dst); dis = clip(deg,1)^-0.5
    out[dst] += h[src] * dis[src] * dis[dst]     # scatter-add over edges
    out += bias

Factorization used here (exact):
    h' = (x @ W.T) * dis[:, None]
    out[d] = dis[d] * sum_{e: dst(e)=d} h'[src(e)]  + bias

Sharding: output rows (and the edge scatter) are partitioned across 8 cores
by contiguous dst blocks of n/8 rows. Every core redundantly computes the
full h' (cheap: 1.6 GFLOP) so the per-edge gather h'[src] is core-local.

Device algorithm per core:
  phase 1: tile over nodes; matmul xT_tile.T @ W.T -> psum; scale rows by
           dis -> h' rows written to a scratch DRAM table (row = 512 B).
  phase 2: edges (sorted by dst, padded) in tiles of 128; chunked dma_gather
           of h'[src] rows into SBUF [128, k, 128]; per tile build a one-hot
           [edge, dst-in-window] matrix with a DVE is_equal against an iota
           row; PE matmul accumulates onehot.T @ msgs into a [128,128] PSUM
           window. dma_gather indices are int16, so the h' table is split
           into two <32768-row halves and phase 2 runs as two passes (pass A
           gathers from the low half, accumulating windows into an SBUF
           accumulator; pass B gathers from the high half and merges).
           Window flush: scale by dis[dst], add bias, DMA out.

Host only does index preprocessing (sort/partition/padding, degree counts
-> dis table) and data layout; all float tensor math runs on device.
"""

import os
from contextlib import ExitStack

import numpy as np

import concourse.bacc as bacc
import concourse.bass as bass
import concourse.mybir as mybir
import concourse.tile as tile
from concourse.bass_utils import run_bass_kernel_spmd

P = 128
N_CORES = 8
F32 = mybir.dt.float32
I16 = mybir.dt.int16
GATHER_K = 8             # tiles per dma_gather (1024 idxs; >1024 faults HW)


def _wrap_idx16(linear):
    """dma_gather index layout: j -> [j%16 + 16c, j//16], replicated c=0..7."""
    num = len(linear)
    assert num % 128 == 0
    assert linear.max(initial=0) < 32768
    cols = num // 16
    a = np.zeros((128, cols), np.int16)
    j = np.arange(num)
    lin16 = linear.astype(np.int16)
    for c in range(8):
        a[j % 16 + 16 * c, j // 16] = lin16
    return a


# ---------------------------------------------------------------- host prep
def _preprocess(edge_index, n, m):
    """Index-only preprocessing: per-core gather/one-hot tables."""
    src = np.asarray(edge_index[0], dtype=np.int64)
    dst = np.asarray(edge_index[1], dtype=np.int64)
    src = np.concatenate([src, np.arange(n, dtype=np.int64)])
    dst = np.concatenate([dst, np.arange(n, dtype=np.int64)])
    deg = np.bincount(dst, minlength=n).astype(np.float64)
    dis = (np.clip(deg, 1.0, None) ** -0.5).astype(np.float32)

    rows_per_core = n // m
    n_win = (rows_per_core + P - 1) // P
    npad = ((n + P - 1) // P) * P
    split = npad // 2                 # h' table halves; each must be < 32768
    assert split < 32768 and npad - split < 32768

    order = np.argsort(dst, kind="stable")
    src_s, dst_s = src[order], dst[order]
    core_of = dst_s // rows_per_core
    local = dst_s - core_of * rows_per_core
    win_of = local // P
    half_of = (src_s >= split).astype(np.int64)

    counts = np.zeros((2, m, n_win), dtype=np.int64)
    np.add.at(counts, (half_of, core_of, win_of), 1)
    C = [int(np.ceil(counts[h].max() / P)) for h in (0, 1)]
    T = [n_win * C[h] for h in (0, 1)]
    TA = [((t + GATHER_K - 1) // GATHER_K) * GATHER_K for t in T]

    per_core = []
    for c in range(m):
        mask_c = core_of == c
        src_c, loc_c = src_s[mask_c], local[mask_c]
        win_c, half_c = win_of[mask_c], half_of[mask_c]
        gidx, dstf = [], []
        for h in (0, 1):
            g = np.zeros((P, T[h]), dtype=np.int32)      # pad idx = 0 (valid row)
            f = np.full((P, T[h]), -1.0, dtype=np.float32)
            for w in range(n_win):
                mw = (win_c == w) & (half_c == h)
                s_w = src_c[mw] - h * split
                l_w = (loc_c[mw] - w * P).astype(np.float32)
                cnt = len(s_w)
                tiles = np.arange(cnt) // P + w * C[h]
                slots = np.arange(cnt) % P
                g[slots, tiles] = s_w
                f[slots, tiles] = l_w
            gidx.append(g)
            dstf.append(f)
        # int16 wrapped gather indices, per pass, chunk layout baked in:
        # chunk ch covers tiles [ch*K, ...): linear j = i*128 + p.
        # All chunks are full GATHER_K tiles (pad tiles gather row 0).
        idx16 = []
        for h in (0, 1):
            parts = []
            g_pad = np.zeros((P, TA[h]), dtype=np.int32)
            g_pad[:, :T[h]] = gidx[h]
            for ch in range(TA[h] // GATHER_K):
                lin = g_pad[:, ch * GATHER_K:(ch + 1) * GATHER_K].T.reshape(-1)
                parts.append(_wrap_idx16(lin))
            idx16.append(np.concatenate(parts, axis=1))
        base = c * rows_per_core
        idx = base + np.arange(n_win * P)
        dl = np.zeros(n_win * P, dtype=np.float32)
        valid = idx < base + rows_per_core
        dl[valid] = dis[idx[valid]]
        disloc = np.ascontiguousarray(dl.reshape(n_win, P).T)
        per_core.append(dict(
            idx16=np.concatenate(idx16, axis=1),          # [128, 8*(T0+T1)]
            dstf=np.concatenate(dstf, axis=1),            # [128, T0+T1]
            disloc=disloc))

    n_t1 = npad // P
    dis_pad = np.zeros(npad, dtype=np.float32)
    dis_pad[:n] = dis
    dis_tbl = np.ascontiguousarray(dis_pad.reshape(n_t1, P).T)

    return dict(per_core=per_core, dis_tbl=dis_tbl, C=C, T=T, TA=TA,
                n_win=n_win, rows_per_core=rows_per_core, npad=npad,
                n_t1=n_t1, split=split)


# ------------------------------------------------------------ device program
def build_program(nc, *, d, npad, n_t1, C, T, TA, n_win, rows_per_core, split,
                  x_slab_tiles=32, h_stage=8, use_barrier=True,
                  dbg_ext_h=False, dbg_passes=(0, 1), dbg_max_tiles=None,
                  dbg_skip_phase1=False):
    """Emit the per-core Tile program (tensors referenced by name)."""
    t_all = T[0] + T[1]
    ta_all = TA[0] + TA[1]
    xT = nc.dram_tensor("xT", [P, npad], F32, kind="ExternalInput")
    WT = nc.dram_tensor("WT", [P, d], F32, kind="ExternalInput")
    bias_t = nc.dram_tensor("bias_t", [P, d], F32, kind="ExternalInput")
    iota_in = nc.dram_tensor("iota_in", [P, P], F32, kind="ExternalInput")
    dis_tbl = nc.dram_tensor("dis_tbl", [P, n_t1], F32, kind="ExternalInput")
    idx16 = nc.dram_tensor("idx16", [P, 8 * ta_all], I16, kind="ExternalInput")
    dstf_in = nc.dram_tensor("dstf", [P, t_all], F32, kind="ExternalInput")
    disloc = nc.dram_tensor("disloc", [P, n_win], F32, kind="ExternalInput")
    if dbg_ext_h:
        h_dram = nc.dram_tensor("h_scratch", [npad, d], F32, kind="ExternalInput")
        h_sink = nc.dram_tensor("h_sink", [npad, d], F32, kind="Internal")
    else:
        h_dram = nc.dram_tensor("h_scratch", [npad, d], F32, kind="Internal")
        h_sink = h_dram
    out = nc.dram_tensor("out", [rows_per_core, d], F32, kind="ExternalOutput")

    with tile.TileContext(nc) as tc, ExitStack() as ctx:
        const = ctx.enter_context(tc.tile_pool(name="const", bufs=1))
        xpool = ctx.enter_context(tc.tile_pool(name="xsl", bufs=3))
        hstg = ctx.enter_context(tc.tile_pool(name="hstg", bufs=3))
        gpool = ctx.enter_context(tc.tile_pool(name="gidx", bufs=3))
        mpool = ctx.enter_context(tc.tile_pool(name="msgs", bufs=3))
        ohpool = ctx.enter_context(tc.tile_pool(name="oh", bufs=6))
        opool = ctx.enter_context(tc.tile_pool(name="ob", bufs=4))
        ps1 = ctx.enter_context(tc.tile_pool(name="ps1", bufs=2, space="PSUM"))
        ps2 = ctx.enter_context(tc.tile_pool(name="ps2", bufs=4, space="PSUM"))

        # ---- constants
        wt_sb = const.tile([P, d], F32)
        nc.sync.dma_start(wt_sb[:], WT[:, :])
        bias_sb = const.tile([P, d], F32)
        nc.sync.dma_start(bias_sb[:], bias_t[:, :])
        iota_sb = const.tile([P, P], F32)
        nc.sync.dma_start(iota_sb[:], iota_in[:, :])
        dis_sb = const.tile([P, n_t1], F32)
        nc.sync.dma_start(dis_sb[:], dis_tbl[:, :])
        dstf_sb = const.tile([P, t_all], F32)
        nc.sync.dma_start(dstf_sb[:], dstf_in[:, :])
        disloc_sb = const.tile([P, n_win], F32)
        nc.sync.dma_start(disloc_sb[:], disloc[:, :])
        acc = const.tile([P, n_win * P], F32)             # pass-A window sums

        # ---- phase 1: h' = (x @ W.T) * dis rows -> h_dram
        h_view = h_sink[:, :].rearrange("(t p) f -> t p f", p=P)  # [n_t1, P, d]
        xs = None
        stg = None
        for t in range(0 if dbg_skip_phase1 else n_t1):
            sl, off = divmod(t, x_slab_tiles)
            if off == 0:
                k_sl = min(x_slab_tiles, n_t1 - sl * x_slab_tiles)
                xs = xpool.tile([P, x_slab_tiles * P], F32)
                nc.sync.dma_start(
                    xs[:, :k_sl * P],
                    xT[:, sl * x_slab_tiles * P: sl * x_slab_tiles * P + k_sl * P])
            st, soff = divmod(t, h_stage)
            if soff == 0:
                k_st = min(h_stage, n_t1 - st * h_stage)
                stg = hstg.tile([P, h_stage, d], F32)
            ps = ps1.tile([P, d], F32)
            nc.tensor.matmul(ps[:], lhsT=xs[:, off * P:(off + 1) * P],
                             rhs=wt_sb[:], start=True, stop=True)
            nc.vector.tensor_scalar_mul(stg[:, soff, :], ps[:], dis_sb[:, t:t + 1])
            if soff == k_st - 1:
                dst = h_view[st * h_stage: st * h_stage + k_st, :, :]
                nc.sync.dma_start(dst.rearrange("i p f -> p i f"),
                                  stg[:, :k_st, :])

        if use_barrier:
            tc.strict_bb_all_engine_barrier()

        # ---- phase 2: two passes of gather + one-hot scatter-add
        n_idx = GATHER_K * P
        for hpass in dbg_passes:
            t_pass = T[hpass]
            c_pass = C[hpass]
            tbl_lo = hpass * split
            tbl_rows = split if hpass == 0 else npad - split
            t_off = hpass * T[0]               # column offset in dstf table
            i16_off = 8 * TA[0] if hpass else 0
            h_tbl = h_dram[tbl_lo: tbl_lo + tbl_rows, :]
            msgs = None
            pw = None
            if dbg_max_tiles is not None:
                t_pass = min(t_pass, dbg_max_tiles)
            for t in range(t_pass):
                ch, i = divmod(t, GATHER_K)
                if i == 0:
                    gi = gpool.tile([P, GATHER_K * 8], I16)
                    nc.sync.dma_start(
                        gi[:, :],
                        idx16[:, i16_off + ch * GATHER_K * 8:
                              i16_off + (ch + 1) * GATHER_K * 8])
                    msgs = mpool.tile([P, GATHER_K, d], F32)
                    nc.gpsimd.dma_gather(
                        out_ap=msgs[:, :, :],
                        in_ap=h_tbl,
                        idxs_ap=gi[:, :],
                        num_idxs=n_idx,
                        num_idxs_reg=n_idx,
                        elem_size=d)
                w, j = divmod(t, c_pass)
                oh = ohpool.tile([P, P], F32)
                nc.vector.tensor_tensor(
                    oh[:], dstf_sb[:, t_off + t: t_off + t + 1].to_broadcast([P, P]),
                    iota_sb[:], op=mybir.AluOpType.is_equal)
                if j == 0:
                    pw = ps2.tile([P, d], F32)
                nc.tensor.matmul(pw[:], lhsT=oh[:], rhs=msgs[:, i, :],
                                 start=(j == 0), stop=(j == c_pass - 1))
                if j == c_pass - 1:
                    if hpass == 0:
                        nc.vector.tensor_copy(acc[:, w * P:(w + 1) * P], pw[:])
                    else:
                        ob = opool.tile([P, d], F32)
                        nc.vector.tensor_add(ob[:], pw[:], acc[:, w * P:(w + 1) * P])
                        nc.vector.scalar_tensor_tensor(
                            ob[:], ob[:], disloc_sb[:, w:w + 1], bias_sb[:],
                            op0=mybir.AluOpType.mult, op1=mybir.AluOpType.add)
                        lo = w * P
                        rows = min(P, rows_per_core - lo)
                        nc.sync.dma_start(out[lo:lo + rows, :], ob[:rows, :])


# ------------------------------------------------------------------- driver
def kernel(x, edge_index, W, bias):
    x = np.asarray(x, dtype=np.float32)
    edge_index = np.asarray(edge_index)
    W = np.asarray(W, dtype=np.float32)
    bias = np.asarray(bias, dtype=np.float32)
    n, d = x.shape
    assert d == P and W.shape == (d, d)

    pp = _preprocess(edge_index, n, N_CORES)
    npad, n_t1 = pp["npad"], pp["n_t1"]

    xT = np.zeros((P, npad), dtype=np.float32)
    xT[:, :n] = x.T
    WT = np.ascontiguousarray(W.T)                       # [din, dout]
    bias_t = np.ascontiguousarray(np.broadcast_to(bias[None, :], (P, d)))
    iota = np.ascontiguousarray(
        np.broadcast_to(np.arange(P, dtype=np.float32)[None, :], (P, P)))

    nc = bacc.Bacc("TRN2", target_bir_lowering=False, debug=False,
                   enable_asserts=False, num_devices=N_CORES)
    build_program(nc, d=d, npad=npad, n_t1=n_t1, C=pp["C"], T=pp["T"],
                  TA=pp["TA"], n_win=pp["n_win"],
                  rows_per_core=pp["rows_per_core"], split=pp["split"])
    nc.compile()

    shared = dict(xT=xT, WT=WT, bias_t=bias_t, iota_in=iota,
                  dis_tbl=pp["dis_tbl"])
    in_maps = []
    for c in range(N_CORES):
        tbl = pp["per_core"][c]
        in_maps.append(dict(shared, idx16=tbl["idx16"], dstf=tbl["dstf"],
                            disloc=tbl["disloc"]))

    res = run_bass_kernel_spmd(nc, in_maps, core_ids=list(range(N_CORES)))
    global LAST_RESULTS, LAST_NC, LAST_IN_MAPS
    LAST_RESULTS, LAST_NC, LAST_IN_MAPS = res, nc, in_maps
    out = np.concatenate([res.results[c]["out"] for c in range(N_CORES)], axis=0)
    return out.astype(np.float32)


LAST_RESULTS = None
LAST_NC = None
LAST_IN_MAPS = None
